# revision 27
# baseline (speedup 1.0000x reference)
"""Expert-parallel MoE GLU MLP kernel for Trainium2.

8 experts -> 8 NeuronCores, one expert per core (no collectives needed).
Per core:  x (C,H) @ w_gate_up (H,2I) -> GLU -> (C,I) @ w_down (I,H) -> (C,H)

Strategy (per core), all matmuls bf16 (fp32 accumulate in PSUM), N=512
moving operands throughout (the hardware max; LDWEIGHTS fully hidden):
  - Host pre-casts x (transposed to (H,C)), w_gate_up, w_down to bf16 and
    reads the output back as bf16: the device does zero dtype conversion
    and weight DMA bytes are halved vs f32.  Total DMA ~54 MB under
    ~663 us of PE work -> fully compute-bound (96% PE occupancy).
    (fp8 was evaluated and rejected: e4m3 costs 6.5% rel err vs the 2e-2
    gate even though DoubleRow would be ~1.4x faster.)
  - PE warm-up: junk matmuls on a zeroed tile fill the HAM clock-gate
    activity window while the initial x/w DMAs land, so real chains run
    at 2.4 GHz from the start.
  - GEMM1: stationary = w_gate_up column tiles loaded in 256-col pairs
    (512 B DMA chunks = SDMA line rate; first/last blocks solo to
    minimize the first chain's critical bytes), moving = xT in 4 quarter
    tiles.  Initial DMAs ride one queue in exact consumption order (the
    start is HBM-bandwidth-bound; splitting across queues regressed).
    GLU = silu(gate) [ACT] * up [DVE] into a bf16 SBUF-resident act
    tile (I, C) - no DRAM round-trip.
  - GEMM2: stationary = act tiles [128i x 128c], moving = w_down h-slabs
    of 512 cols -> psum accumulates the full 32-tile I chain -> cast ->
    out (C,H) bf16.  Slab 0 is prefetched during phase 1; slab hc+1 at
    the start of slab hc's chains.  Out-DMAs ride the scalar HWDGE queue
    so slab prefetches never queue behind them.  The final tile drains
    as two half-width chains to shorten the kernel tail.
  - Single PSUM pool, 8 banks exactly (warmup shares the pg rotation);
    all matmuls one dtype (avoids the dtype-interleave FWL hazard).
"""
import numpy as np
import ml_dtypes

E, C, H, I = 8, 1024, 2048, 4096
P = 128
HT, IT, CT = H // P, I // P, C // P  # 16, 32, 8
NP1 = IT // 2         # 16 gate/up weight pair-blocks (256 cols each)
NHC = 4               # w_down h-slabs
HW_ = H // NHC        # 512 cols per slab
WARM = 40             # junk matmuls to warm the PE clock gate
# phase-1 weight blocks: first/last solo (128 cols) so the first chain
# unblocks on minimum bytes, pairs (256 cols = 512 B DMA lines) between
BLOCKS = [(0, 1)] + [(1 + 2 * k, 2) for k in range(15)] + [(31, 1)]

_CACHE = {}


def _build():
    import concourse.bacc as bacc
    import concourse.mybir as mybir
    import concourse.tile as tile

    f32 = mybir.dt.float32
    bf16 = mybir.dt.bfloat16
    AF = mybir.ActivationFunctionType

    nc = bacc.Bacc("TRN2", target_bir_lowering=False, debug=False)
    xT = nc.declare_dram_parameter("xT", [H, C], bf16, isOutput=False).ap()
    wgu = nc.declare_dram_parameter("wgu", [H, 2 * I], bf16, isOutput=False).ap()
    wdn = nc.declare_dram_parameter("wdn", [I, H], bf16, isOutput=False).ap()
    out = nc.declare_dram_parameter("out", [C, H], bf16, isOutput=True).ap()

    xT_v = xT.rearrange("(ht p) c -> p ht c", p=P)    # [128, 16, 1024]
    wgu_v = wgu.rearrange("(ht p) f -> p ht f", p=P)  # [128, 16, 8192]
    wdn_v = wdn.rearrange("(it p) h -> p it h", p=P)  # [128, 32, 2048]
    out_v = out.rearrange("(ct p) h -> p ct h", p=P)  # [128, 8, 2048]

    with tile.TileContext(nc) as tc:
        with (
            tc.tile_pool(name="acts_pool", bufs=1) as actsp,
            tc.tile_pool(name="pp", bufs=1, space="PSUM") as pp,
            tc.tile_pool(name="sbs", bufs=3) as sbs,
            tc.tile_pool(name="w2pre", bufs=1) as w2pre,
        ):
            # acts[p, it, c] = act row (it*128+p), col c  (bf16, 8 MB)
            acts = actsp.tile([P, IT, C], bf16)
            # slab 0 of w_down, prefetched during phase 1
            wsl0 = w2pre.tile([P, IT, HW_], bf16, name="wsl0")

            # ---- Phase 1: gate_up GEMM + GLU -> acts ---------------------
            with (
                tc.tile_pool(name="xt_pool", bufs=1) as xtp,
                tc.tile_pool(name="w1", bufs=2) as w1,
            ):
                # PE warm-up on a zeroed tile (no data deps -> runs during
                # the initial DMAs; ~2.5 us of junk matmuls)
                wtile = sbs.tile([P, P], bf16, tag="warm", bufs=1)
                nc.vector.memset(wtile, 0.0)
                # warm-up PSUM shares the pg tag rotation (stays at 8 banks)
                pw = pp.tile([P, 512], f32, tag="pg", bufs=2)
                for w in range(WARM):
                    nc.tensor.matmul(pw[:, 0:P], wtile, wtile,
                                     start=True, stop=True)

                # x as 4 separate tiles (per c-half x ht-half) so each
                # chain half depends only on its own DMA, not the whole
                # 4 MB of x
                xq = [[xtp.tile([P, HT // 2, 512], bf16, name=f"xq{cc}{hh}")
                       for hh in range(2)] for cc in range(2)]

                def xmov(cc, ht):
                    return xq[cc][ht // 8][:, ht % 8, :]

                def load_block(b):
                    i0, ni = BLOCKS[b]
                    fs = slice(i0 * P, (i0 + ni) * P)
                    us = slice(I + i0 * P, I + (i0 + ni) * P)
                    wg = w1.tile([P, HT, 2 * P], bf16, tag="wg", name=f"wg{b}")
                    wu = w1.tile([P, HT, 2 * P], bf16, tag="wu", name=f"wu{b}")
                    if b == 0:
                        # single queue, exact consumption order: chain 0's
                        # first half-group unblocks after 1.25 MB
                        h0, h1 = slice(0, 8), slice(8, 16)
                        nc.sync.dma_start(wg[:, h0, 0:P], wgu_v[:, h0, fs])
                        nc.sync.dma_start(xq[0][0], xT_v[:, 0:8, 0:512])
                        nc.sync.dma_start(wg[:, h1, 0:P], wgu_v[:, h1, fs])
                        nc.sync.dma_start(xq[0][1], xT_v[:, 8:16, 0:512])
                        nc.sync.dma_start(wu[:, :, 0:P], wgu_v[:, :, us])
                        nc.sync.dma_start(xq[1][0], xT_v[:, 0:8, 512:1024])
                        nc.sync.dma_start(xq[1][1], xT_v[:, 8:16, 512:1024])
                    else:
                        nc.sync.dma_start(wg[:, :, 0:ni * P], wgu_v[:, :, fs])
                        nc.sync.dma_start(wu[:, :, 0:ni * P], wgu_v[:, :, us])
                    return wg, wu

                w_next = load_block(0)
                for b in range(len(BLOCKS)):
                    wgp, wup = w_next
                    if b + 1 < len(BLOCKS):
                        w_next = load_block(b + 1)
                    if 3 <= b < 7:
                        # slab-0 quarter loads, spread mid-phase-1
                        q = b - 3
                        qs = slice(q * (IT // 4), (q + 1) * (IT // 4))
                        nc.sync.dma_start(wsl0[:, qs, :], wdn_v[:, qs, 0:HW_])
                    i0, ni = BLOCKS[b]
                    for k in range(ni):
                        i = i0 + k
                        ks = slice(k * P, (k + 1) * P)
                        for cc in range(2):
                            cs = slice(cc * 512, (cc + 1) * 512)
                            pg = pp.tile([P, 512], f32, tag="pg", bufs=2)
                            pu = pp.tile([P, 512], f32, tag="pu", bufs=2)
                            # two accumulation groups per chain (ht 0-7
                            # and 8-15 into the same bank): the head wait
                            # of group 1 covers only the first half's
                            # inputs, so the chain starts before the
                            # second x quarter has landed
                            for ht in range(8):
                                nc.tensor.matmul(
                                    pg, wgp[:, ht, ks], xmov(cc, ht),
                                    start=(ht == 0), stop=False)
                            for ht in range(8, HT):
                                nc.tensor.matmul(
                                    pg, wgp[:, ht, ks], xmov(cc, ht),
                                    start=False, stop=(ht == HT - 1))
                            for ht in range(8):
                                nc.tensor.matmul(
                                    pu, wup[:, ht, ks], xmov(cc, ht),
                                    start=(ht == 0), stop=False)
                            for ht in range(8, HT):
                                nc.tensor.matmul(
                                    pu, wup[:, ht, ks], xmov(cc, ht),
                                    start=False, stop=(ht == HT - 1))
                            sil = sbs.tile([P, 512], f32, tag="sil")
                            nc.scalar.activation(sil, pg, AF.Silu)
                            nc.vector.tensor_mul(acts[:, i, cs], sil, pu)

            # ---- Phase 2: down GEMM (bf16), full-I psum chains -----------
            with tc.tile_pool(name="w2", bufs=2) as w2:

                def load_slab(hc):
                    hs = slice(hc * HW_, (hc + 1) * HW_)
                    wsl = w2.tile([P, IT, HW_], bf16, tag="wsl", name=f"ws{hc}")
                    for q in range(4):
                        qs = slice(q * (IT // 4), (q + 1) * (IT // 4))
                        nc.sync.dma_start(wsl[:, qs, :], wdn_v[:, qs, hs])
                    return wsl

                wsl_cur = wsl0
                wsl_next = load_slab(1)
                for hc in range(NHC):
                    hs = slice(hc * HW_, (hc + 1) * HW_)
                    for ct in range(CT):
                        last = hc == NHC - 1 and ct == CT - 1
                        if not last:
                            ps = pp.tile([P, HW_], f32, tag="ps", bufs=2)
                            for i in range(IT):
                                nc.tensor.matmul(
                                    ps,
                                    acts[:, i, ct * P:(ct + 1) * P],
                                    wsl_cur[:, i, :],
                                    start=(i == 0), stop=(i == IT - 1))
                            osb = sbs.tile([P, HW_], bf16, tag="osb")
                            nc.vector.tensor_copy(osb, ps)
                            # out-DMAs ride the scalar HWDGE queue so slab
                            # prefetches on sync never queue behind them
                            nc.scalar.dma_start(out_v[:, ct, hs], osb)
                        else:
                            # final tile: two half-width chains so the first
                            # half's drain overlaps the second half's MMs
                            for g in range(2):
                                gs = slice(hc * HW_ + g * (HW_ // 2),
                                           hc * HW_ + (g + 1) * (HW_ // 2))
                                wl = slice(g * (HW_ // 2), (g + 1) * (HW_ // 2))
                                ps = pp.tile([P, HW_ // 2], f32,
                                             tag="psl", bufs=2)
                                for i in range(IT):
                                    nc.tensor.matmul(
                                        ps,
                                        acts[:, i, ct * P:(ct + 1) * P],
                                        wsl_cur[:, i, wl],
                                        start=(i == 0), stop=(i == IT - 1))
                                osb = sbs.tile([P, HW_ // 2], bf16, tag="osl")
                                nc.vector.tensor_copy(osb, ps)
                                nc.scalar.dma_start(out_v[:, ct, gs], osb)
                    if hc + 1 < NHC:
                        wsl_cur = wsl_next
                        if hc + 2 < NHC:
                            wsl_next = load_slab(hc + 2)

    nc.compile()
    return nc


def _get_nc():
    if "nc" not in _CACHE:
        _CACHE["nc"] = _build()
    return _CACHE["nc"]


def _bf16(a):
    return np.ascontiguousarray(np.asarray(a, dtype=ml_dtypes.bfloat16))


def _run(hidden_states, w_gate_up, w_down, trace=False):
    from concourse.bass_utils import run_bass_kernel_spmd

    nc = _get_nc()
    hs = np.asarray(hidden_states, dtype=np.float32)
    in_maps = [
        {
            "xT": _bf16(hs[e].T),
            "wgu": _bf16(w_gate_up[e]),
            "wdn": _bf16(w_down[e]),
        }
        for e in range(E)
    ]
    res = run_bass_kernel_spmd(nc, in_maps, list(range(E)), trace=trace)
    output = np.stack(
        [np.asarray(res.results[e]["out"], dtype=np.float32) for e in range(E)],
        axis=0)
    return output, res


def kernel(hidden_states, w_gate_up, w_down):
    output, _ = _run(hidden_states, w_gate_up, w_down, trace=False)
    return output
